# revision 4
# baseline (speedup 1.0000x reference)
"""Trainium2 Bass kernel for nn_AttentionInPnts (sparse local attention over points).

Math (per batch b, point n):
  q = wq @ xc, k_j = wk @ x_j, v_j = wv @ x_j   (x_16 == xc, the center)
  logit_j = (q . k_j) / 8 = xc^T (wq^T wk) x_j / 8 = y . x_j / 8,  y = A^T-contracted center
  a = softmax(logit)                            (17 entries)
  out = sum_j a_j v_j = wv @ (sum_j a_j x_j)    (projection commutes with the
                                                 scalar-weighted sum)

Per point-tile of 128 points: one small PE matmul for y, one DVE mul +
reduce for the 17 logits, a softmax on [128, 17], then the TensorEngine
computes the weighted sum via the diagonal-rhs trick:
  s[c, p] = sum_j matmul(lhsT = x_j[p', c], rhs = diag(a_j)[p', p])  (PSUM accum)
and one final matmul applies wv.

Host-side prep (cheap, numpy): concatenate near+center into one tensor (one
DMA per tile -> single semaphore wait for the TensorEngine), pre-transpose
the center features, precompute A = wq^T wk and wv^T.

Sharding: pure data-parallel, batch b -> core b (8 batches, 8 cores).
"""

import os

import numpy as np

BS = 8
NPTS = 4096
KNB = 16
C = 128
J = KNB + 1  # 16 near + 1 center
P = 128  # points per tile
NTILES = NPTS // P
SCALE = 1.0 / 8.0  # 1/sqrt(c//2)

_cache = {}

# set by kernel() when tracing is enabled (BASS_KERNEL_TRACE=1)
last_exec_ns = None
last_results = None


def _build():
    import concourse.bass as bass
    import concourse.tile as tile
    from concourse import bacc, mybir

    f32 = mybir.dt.float32
    nc = bacc.Bacc()

    xfull = nc.declare_dram_parameter("xfull", [NPTS, J, C], f32, isOutput=False)
    fcT = nc.declare_dram_parameter("fcT", [C, NPTS], f32, isOutput=False)
    amat = nc.declare_dram_parameter("amat", [C, C], f32, isOutput=False)
    wvt = nc.declare_dram_parameter("wvt", [C, C], f32, isOutput=False)
    irepj = nc.declare_dram_parameter("irepj", [P, J, P], f32, isOutput=False)
    out = nc.declare_dram_parameter("out", [NPTS, C], f32, isOutput=True)

    with tile.TileContext(nc) as tc:
        with (
            tc.tile_pool(name="consts", bufs=1) as consts,
            tc.tile_pool(name="big", bufs=2) as big,
            tc.tile_pool(name="small", bufs=3) as small,
            tc.tile_pool(name="psum", bufs=2, space="PSUM") as psum,
        ):
            amat_sb = consts.tile([C, C], f32)
            nc.sync.dma_start(out=amat_sb, in_=amat[:])
            wvt_sb = consts.tile([C, C], f32)
            nc.sync.dma_start(out=wvt_sb, in_=wvt[:])
            irepj_sb = consts.tile([P, J, P], f32)
            nc.sync.dma_start(out=irepj_sb, in_=irepj[:])

            for it in range(NTILES):
                r0 = it * P
                # xa = [p, j, c] with j=0..15 near, j=16 center
                xa = big.tile([P, J, C], f32)
                nc.sync.dma_start(out=xa[:], in_=xfull[r0 : r0 + P, :, :])
                # center transposed [c, p], from host
                xcT = small.tile([C, P], f32)
                nc.sync.dma_start(out=xcT[:], in_=fcT[:, r0 : r0 + P])

                # y[p, c'] = sum_c xc[p, c] * A[c, c']
                y_ps = psum.tile([P, C], f32)
                nc.tensor.matmul(y_ps, lhsT=xcT[:], rhs=amat_sb[:], start=True, stop=True)
                y_sb = small.tile([P, C], f32)
                nc.scalar.copy(y_sb, y_ps)

                # t[p, j, c] = xa * y (y broadcast over j)
                y_ap = y_sb[:]
                y_bc = bass.AP(
                    tensor=y_ap.tensor,
                    offset=y_ap.offset,
                    ap=[y_ap.ap[0], [0, J], y_ap.ap[1]],
                )
                t = big.tile([P, J, C], f32)
                nc.vector.tensor_tensor(out=t[:], in0=xa[:], in1=y_bc, op=mybir.AluOpType.mult)

                # logits L[p, j] = sum_c t
                logit = small.tile([P, J], f32)
                nc.vector.tensor_reduce(
                    out=logit[:], in_=t[:], axis=mybir.AxisListType.X, op=mybir.AluOpType.add
                )

                # e = exp(L/8), sum_e = sum_j e
                e_sb = small.tile([P, J], f32)
                sum_e = small.tile([P, 1], f32)
                nc.scalar.activation(
                    out=e_sb[:],
                    in_=logit[:],
                    func=mybir.ActivationFunctionType.Exp,
                    scale=SCALE,
                    accum_out=sum_e[:],
                )
                inv = small.tile([P, 1], f32)
                nc.vector.reciprocal(inv[:], sum_e[:])
                a_sb = small.tile([P, J], f32)
                nc.vector.tensor_scalar_mul(a_sb[:], e_sb[:], inv[:])

                # diag[p', j, p] = (p' == p) * a[p', j]
                a_ap = a_sb[:]
                a_bc = bass.AP(
                    tensor=a_ap.tensor,
                    offset=a_ap.offset,
                    ap=[a_ap.ap[0], a_ap.ap[1], [0, P]],
                )
                diag = big.tile([P, J, P], f32)
                nc.vector.tensor_tensor(
                    out=diag[:], in0=irepj_sb[:], in1=a_bc, op=mybir.AluOpType.mult
                )

                # s[c, p] = sum_j x_j[p', c]^T @ diag_j[p', p]  (PSUM accumulate)
                s_ps = psum.tile([C, P], f32)
                for j in range(J):
                    nc.tensor.matmul(
                        s_ps,
                        lhsT=xa[:, j, :],
                        rhs=diag[:, j, :],
                        start=(j == 0),
                        stop=(j == J - 1),
                    )
                s_sb = small.tile([C, P], f32)
                nc.scalar.copy(s_sb, s_ps)

                # o[p, c'] = sum_c s[c, p] * wvT[c, c']
                o_ps = psum.tile([P, C], f32)
                nc.tensor.matmul(o_ps, lhsT=s_sb[:], rhs=wvt_sb[:], start=True, stop=True)
                o_sb = small.tile([P, C], f32)
                nc.scalar.copy(o_sb, o_ps)

                nc.sync.dma_start(out=out[r0 : r0 + P, :], in_=o_sb[:])

    nc.compile()
    return nc


def _get_nc():
    if "nc" not in _cache:
        _cache["nc"] = _build()
    return _cache["nc"]


def kernel(fea_center, fea_near, wq, wk, wv):
    global last_exec_ns, last_results
    from concourse.bass_utils import run_bass_kernel_spmd

    fea_center = np.asarray(fea_center, dtype=np.float32)
    fea_near = np.asarray(fea_near, dtype=np.float32)
    wq = np.asarray(wq, dtype=np.float32)
    wk = np.asarray(wk, dtype=np.float32)
    wv = np.asarray(wv, dtype=np.float32)

    amat = np.ascontiguousarray(wq.T @ wk)  # [c_center, c_near]
    wvt = np.ascontiguousarray(wv.T)  # [c_in, c_out]
    irepj = np.ascontiguousarray(
        np.broadcast_to(np.eye(P, dtype=np.float32)[:, None, :], (P, J, P))
    )

    # [bs, n, 17, c]: near neighbors then the center as the 17th entry
    xfull = np.concatenate([fea_near, fea_center], axis=2)
    # transposed center features [bs, c, n]
    fcT = np.ascontiguousarray(np.transpose(fea_center[:, :, 0, :], (0, 2, 1)))

    nc = _get_nc()
    in_maps = []
    for b in range(BS):
        in_maps.append(
            {
                "xfull": np.ascontiguousarray(xfull[b]),
                "fcT": fcT[b],
                "amat": amat,
                "wvt": wvt,
                "irepj": irepj,
            }
        )

    trace = bool(int(os.environ.get("BASS_KERNEL_TRACE", "0")))
    res = run_bass_kernel_spmd(nc, in_maps, core_ids=list(range(BS)), trace=trace)
    last_exec_ns = res.exec_time_ns
    last_results = res
    out = np.stack([res.results[b]["out"] for b in range(BS)], axis=0)
    return out


# revision 18
# speedup vs baseline: 2.7267x; 2.7267x over previous
"""Trainium2 Bass kernel for nn_AttentionInPnts (sparse local attention over points).

Math (per batch b, point n):
  q = wq @ xc, k_j = wk @ x_j, v_j = wv @ x_j   (x_16 == xc, the center)
  logit_j = (q . k_j) / 8 = xc^T (wq^T wk) x_j / 8 = y . x_j / 8
  a = softmax(logit)                            (17 entries)
  out = sum_j a_j v_j = wv @ (sum_j a_j x_j)    (projection commutes with the
                                                 scalar-weighted sum)

Per point-tile of 128 points: one small PE matmul for y, one DVE mul +
reduce for the 17 logits, a softmax on [128, 17], then the TensorEngine
computes the weighted sum via the diagonal-rhs trick:
  s[c, p] = sum_j matmul(lhsT = x_j[p', c], rhs = diag(a_j)[p', p])  (PSUM accum)
and one final matmul applies wv. The diag tensors are built on the otherwise
idle GpSimd engine with affine_select.

Host-side prep (cheap, numpy): concatenate near+center, pre-transpose the
center, precompute A = wq^T wk and wv^T, and cast the streamed operands to
bf16 (halves HBM traffic; accumulations stay fp32 in PSUM / DVE internals).

Sharding: pure data-parallel, batch b -> core b (8 batches, 8 cores).
"""

import os

import numpy as np

BS = 8
NPTS = 4096
KNB = 16
C = 128
J = KNB + 1  # 16 near + 1 center
P = 128  # points per tile
NTILES = NPTS // P
SCALE = 1.0 / 8.0  # 1/sqrt(c//2)

_cache = {}

# set by kernel() when tracing is enabled (BASS_KERNEL_TRACE=1)
last_exec_ns = None
last_results = None


def _build():
    import concourse.bass as bass
    import concourse.tile as tile
    from concourse import bacc, mybir

    f32 = mybir.dt.float32
    bf16 = mybir.dt.bfloat16
    nc = bacc.Bacc()

    xfull = nc.declare_dram_parameter("xfull", [NPTS, J, C], bf16, isOutput=False)
    fcT = nc.declare_dram_parameter("fcT", [C, NPTS], bf16, isOutput=False)
    amat = nc.declare_dram_parameter("amat", [C, C], bf16, isOutput=False)
    wvt = nc.declare_dram_parameter("wvt", [C, C], bf16, isOutput=False)
    out = nc.declare_dram_parameter("out", [NPTS, C], f32, isOutput=True)

    with tile.TileContext(nc) as tc:
        with (
            tc.tile_pool(name="consts", bufs=1) as consts,
            tc.tile_pool(name="big", bufs=4) as big,
            tc.tile_pool(name="small", bufs=6) as small,
            tc.tile_pool(name="psum", bufs=2, space="PSUM") as psum,
        ):
            amat_sb = consts.tile([C, C], bf16)
            nc.sync.dma_start(out=amat_sb, in_=amat[:])
            wvt_sb = consts.tile([C, C], bf16)
            nc.sync.dma_start(out=wvt_sb, in_=wvt[:])

            for it in range(NTILES):
                r0 = it * P
                # xa = [p, j, c] bf16 with j=0..15 near, j=16 center
                # (two dma_starts -> two HW queues in flight; Bacc splits the
                # multi-sem waits for PE consumers)
                xa = big.tile([P, J, C], bf16)
                nc.sync.dma_start(out=xa[:], in_=xfull[r0 : r0 + P, :, :])
                # center transposed [c, p] bf16, from host
                xcT = small.tile([C, P], bf16)
                nc.sync.dma_start(out=xcT[:], in_=fcT[:, r0 : r0 + P])

                # y[p, c'] = sum_c xc[p, c] * A[c, c']  (fp32 accumulate)
                y_ps = psum.tile([P, C], f32)
                nc.tensor.matmul(y_ps, lhsT=xcT[:], rhs=amat_sb[:], start=True, stop=True)
                y_sb = small.tile([P, C], bf16)
                nc.scalar.copy(y_sb, y_ps)

                # t[p, j, c] = xa * y (y broadcast over j), bf16 2x mode
                y_ap = y_sb[:]
                y_bc = bass.AP(
                    tensor=y_ap.tensor,
                    offset=y_ap.offset,
                    ap=[y_ap.ap[0], [0, J], y_ap.ap[1]],
                )
                t = big.tile([P, J, C], bf16)
                nc.vector.tensor_tensor(out=t[:], in0=xa[:], in1=y_bc, op=mybir.AluOpType.mult)

                # logits L[p, j] = sum_c t: two pairwise folds (bf16, 2x mode)
                # then a short fp32 reduce over the remaining 32 lanes
                f1 = small.tile([P, J, C // 2], bf16)
                nc.vector.tensor_tensor(
                    out=f1[:], in0=t[:, :, 0 : C // 2], in1=t[:, :, C // 2 : C],
                    op=mybir.AluOpType.add,
                )
                f2 = small.tile([P, J, C // 4], bf16)
                nc.vector.tensor_tensor(
                    out=f2[:], in0=f1[:, :, 0 : C // 4], in1=f1[:, :, C // 4 : C // 2],
                    op=mybir.AluOpType.add,
                )
                f3 = small.tile([P, J, C // 8], bf16)
                nc.vector.tensor_tensor(
                    out=f3[:], in0=f2[:, :, 0 : C // 8], in1=f2[:, :, C // 8 : C // 4],
                    op=mybir.AluOpType.add,
                )
                logit = small.tile([P, J], f32)
                nc.vector.tensor_reduce(
                    out=logit[:], in_=f3[:], axis=mybir.AxisListType.X, op=mybir.AluOpType.add
                )

                # e = exp(L/8) in bf16, sum_e = sum_j e in fp32; the softmax
                # normalization is folded into the final output copy (x 1/sum_e)
                e_sb = small.tile([P, J], bf16)
                sum_e = small.tile([P, 1], f32)
                nc.scalar.activation(
                    out=e_sb[:],
                    in_=logit[:],
                    func=mybir.ActivationFunctionType.Exp,
                    scale=SCALE,
                    accum_out=sum_e[:],
                )
                inv = small.tile([P, 1], f32)
                nc.vector.reciprocal(inv[:], sum_e[:])

                # diag[p', j, p] = (p' == p) * e[p', j]   (GpSimd affine_select)
                a_ap = e_sb[:]
                diag = big.tile([P, J, P], bf16)
                JSPLIT = 9
                nc.gpsimd.affine_select(
                    out=diag[:, 0:JSPLIT, :],
                    in_=bass.AP(tensor=a_ap.tensor, offset=a_ap.offset,
                                ap=[a_ap.ap[0], [a_ap.ap[1][0], JSPLIT], [0, P]]),
                    compare_op=mybir.AluOpType.is_equal,
                    fill=0.0,
                    base=0,
                    # iota(x, j, p) = x - p; == 0 on the diagonal
                    pattern=[[0, JSPLIT], [-1, P]],
                    channel_multiplier=1,
                )
                nc.gpsimd.affine_select(
                    out=diag[:, JSPLIT:J, :],
                    in_=bass.AP(tensor=a_ap.tensor,
                                offset=a_ap.offset + JSPLIT * a_ap.ap[1][0],
                                ap=[a_ap.ap[0], [a_ap.ap[1][0], J - JSPLIT], [0, P]]),
                    compare_op=mybir.AluOpType.is_equal,
                    fill=0.0,
                    base=0,
                    pattern=[[0, J - JSPLIT], [-1, P]],
                    channel_multiplier=1,
                )

                # s[c, p] = sum_j x_j[p', c]^T @ diag_j[p', p]  (PSUM accumulate)
                s_ps = psum.tile([C, P], f32)
                for j in range(J):
                    nc.tensor.matmul(
                        s_ps,
                        lhsT=xa[:, j, :],
                        rhs=diag[:, j, :],
                        start=(j == 0),
                        stop=(j == J - 1),
                    )
                s_sb = small.tile([C, P], bf16)
                nc.scalar.copy(s_sb, s_ps)

                # o[p, c'] = (sum_c s[c, p] * wvT[c, c']) / sum_e[p]
                o_ps = psum.tile([P, C], f32)
                nc.tensor.matmul(o_ps, lhsT=s_sb[:], rhs=wvt_sb[:], start=True, stop=True)
                o_sb = small.tile([P, C], f32)
                nc.scalar.mul(o_sb, o_ps, inv[:])

                nc.sync.dma_start(out=out[r0 : r0 + P, :], in_=o_sb[:])

    nc.compile()
    return nc


def _get_nc():
    if "nc" not in _cache:
        _cache["nc"] = _build()
    return _cache["nc"]


def kernel(fea_center, fea_near, wq, wk, wv):
    global last_exec_ns, last_results
    import ml_dtypes

    from concourse.bass_utils import run_bass_kernel_spmd

    bf = ml_dtypes.bfloat16
    fea_center = np.asarray(fea_center, dtype=np.float32)
    fea_near = np.asarray(fea_near, dtype=np.float32)
    wq = np.asarray(wq, dtype=np.float32)
    wk = np.asarray(wk, dtype=np.float32)
    wv = np.asarray(wv, dtype=np.float32)

    amat = np.ascontiguousarray(wq.T @ wk).astype(bf)  # [c_center, c_near]
    wvt = np.ascontiguousarray(wv.T).astype(bf)  # [c_in, c_out]

    # [bs, n, 17, c]: near neighbors then the center as the 17th entry
    xfull = np.concatenate([fea_near, fea_center], axis=2).astype(bf)
    # transposed center features [bs, c, n]
    fcT = np.ascontiguousarray(np.transpose(fea_center[:, :, 0, :], (0, 2, 1))).astype(bf)

    nc = _get_nc()
    in_maps = []
    for b in range(BS):
        in_maps.append(
            {
                "xfull": np.ascontiguousarray(xfull[b]),
                "fcT": np.ascontiguousarray(fcT[b]),
                "amat": amat,
                "wvt": wvt,
            }
        )

    trace = bool(int(os.environ.get("BASS_KERNEL_TRACE", "0")))
    res = run_bass_kernel_spmd(nc, in_maps, core_ids=list(range(BS)), trace=trace)
    last_exec_ns = res.exec_time_ns
    last_results = res
    out = np.stack([res.results[b]["out"] for b in range(BS)], axis=0)
    return out


# revision 19
# speedup vs baseline: 2.9728x; 1.0903x over previous
"""Trainium2 Bass kernel for nn_AttentionInPnts (sparse local attention over points).

Math (per batch b, point n):
  q = wq @ xc, k_j = wk @ x_j, v_j = wv @ x_j   (x_16 == xc, the center)
  logit_j = (q . k_j) / 8 = xc^T (wq^T wk) x_j / 8 = y . x_j / 8
  a = softmax(logit)                            (17 entries)
  out = sum_j a_j v_j = wv @ (sum_j a_j x_j)    (projection commutes with the
                                                 scalar-weighted sum)

Per point-tile of 128 points: one small PE matmul for y, one DVE mul +
reduce for the 17 logits, a softmax on [128, 17], then the TensorEngine
computes the weighted sum via the diagonal-rhs trick:
  s[c, p] = sum_j matmul(lhsT = x_j[p', c], rhs = diag(a_j)[p', p])  (PSUM accum)
and one final matmul applies wv. The diag tensors are built on the otherwise
idle GpSimd engine with affine_select.

Host-side prep (cheap, numpy): concatenate near+center, pre-transpose the
center, precompute A = wq^T wk and wv^T, and cast the streamed operands to
bf16 (halves HBM traffic; accumulations stay fp32 in PSUM / DVE internals).

Sharding: pure data-parallel, batch b -> core b (8 batches, 8 cores).
"""

import os

import numpy as np

BS = 8
NPTS = 4096
KNB = 16
C = 128
J = KNB + 1  # 16 near + 1 center
P = 128  # points per tile
NTILES = NPTS // P
SCALE = 1.0 / 8.0  # 1/sqrt(c//2)

_cache = {}

# set by kernel() when tracing is enabled (BASS_KERNEL_TRACE=1)
last_exec_ns = None
last_results = None


def _build():
    import concourse.bass as bass
    import concourse.tile as tile
    from concourse import bacc, mybir

    f32 = mybir.dt.float32
    bf16 = mybir.dt.bfloat16
    nc = bacc.Bacc()

    xfull = nc.declare_dram_parameter("xfull", [NPTS, J, C], bf16, isOutput=False)
    fcT = nc.declare_dram_parameter("fcT", [C, NPTS], bf16, isOutput=False)
    amat = nc.declare_dram_parameter("amat", [C, C], bf16, isOutput=False)
    wvt = nc.declare_dram_parameter("wvt", [C, C], bf16, isOutput=False)
    i16 = mybir.dt.int16
    sidx0 = nc.declare_dram_parameter("sidx0", [P, 8], i16, isOutput=False)
    sidx1 = nc.declare_dram_parameter("sidx1", [P, 10], i16, isOutput=False)
    out = nc.declare_dram_parameter("out", [NPTS, C], f32, isOutput=True)

    with tile.TileContext(nc) as tc:
        with (
            tc.tile_pool(name="consts", bufs=1) as consts,
            tc.tile_pool(name="big", bufs=4) as big,
            tc.tile_pool(name="small", bufs=6) as small,
            tc.tile_pool(name="psum", bufs=2, space="PSUM") as psum,
        ):
            amat_sb = consts.tile([C, C], bf16)
            nc.sync.dma_start(out=amat_sb, in_=amat[:])
            wvt_sb = consts.tile([C, C], bf16)
            nc.sync.dma_start(out=wvt_sb, in_=wvt[:])
            sidx0_sb = consts.tile([P, 8], i16)
            nc.sync.dma_start(out=sidx0_sb, in_=sidx0[:])
            sidx1_sb = consts.tile([P, 10], i16)
            nc.sync.dma_start(out=sidx1_sb, in_=sidx1[:])

            for it in range(NTILES):
                r0 = it * P
                # xa = [p, j, c] bf16 with j=0..15 near, j=16 center
                # (two dma_starts -> two HW queues in flight; Bacc splits the
                # multi-sem waits for PE consumers)
                xa = big.tile([P, J, C], bf16)
                nc.sync.dma_start(out=xa[:], in_=xfull[r0 : r0 + P, :, :])
                # center transposed [c, p] bf16, from host
                xcT = small.tile([C, P], bf16)
                nc.sync.dma_start(out=xcT[:], in_=fcT[:, r0 : r0 + P])

                # y[p, c'] = sum_c xc[p, c] * A[c, c']  (fp32 accumulate)
                y_ps = psum.tile([P, C], f32)
                nc.tensor.matmul(y_ps, lhsT=xcT[:], rhs=amat_sb[:], start=True, stop=True)
                y_sb = small.tile([P, C], bf16)
                nc.scalar.copy(y_sb, y_ps)

                # t[p, j, c] = xa * y (y broadcast over j), bf16 2x mode
                y_ap = y_sb[:]
                y_bc = bass.AP(
                    tensor=y_ap.tensor,
                    offset=y_ap.offset,
                    ap=[y_ap.ap[0], [0, J], y_ap.ap[1]],
                )
                t = big.tile([P, J, C], bf16)
                nc.vector.tensor_tensor(out=t[:], in0=xa[:], in1=y_bc, op=mybir.AluOpType.mult)

                # logits L[p, j] = sum_c t (strided reduce, 1x)
                logit = small.tile([P, J], f32)
                nc.vector.tensor_reduce(
                    out=logit[:], in_=t[:], axis=mybir.AxisListType.X, op=mybir.AluOpType.add
                )

                # e = exp(L/8) in bf16, sum_e = sum_j e in fp32; the softmax
                # normalization is folded into the final output copy (x 1/sum_e)
                e_sb = small.tile([P, J + 1], bf16)
                sum_e = small.tile([P, 1], f32)
                nc.scalar.activation(
                    out=e_sb[:, 0:J],
                    in_=logit[:],
                    func=mybir.ActivationFunctionType.Exp,
                    scale=SCALE,
                    accum_out=sum_e[:],
                )
                inv = small.tile([P, 1], f32)
                nc.vector.reciprocal(inv[:], sum_e[:])

                # diag[p', j, p] = (p' == p) * e[p', j] via GpSimd local_scatter
                # (zeros + per-partition scatter of the 17 diagonal values; the
                # int16 index tables are tile-invariant constants from the host)
                diag = big.tile([P, J, P], bf16)
                nc.gpsimd.local_scatter(
                    out_ap=diag[:, 0:8, :],
                    data_ap=e_sb[:, 0:8],
                    idxs_ap=sidx0_sb[:],
                    channels=P,
                    num_elems=8 * P,
                    num_idxs=8,
                )
                nc.gpsimd.local_scatter(
                    out_ap=diag[:, 8:J, :],
                    data_ap=e_sb[:, 8 : J + 1],
                    idxs_ap=sidx1_sb[:],
                    channels=P,
                    num_elems=9 * P,
                    num_idxs=10,
                )

                # s[c, p] = sum_j x_j[p', c]^T @ diag_j[p', p]  (PSUM accumulate)
                s_ps = psum.tile([C, P], f32)
                for j in range(J):
                    nc.tensor.matmul(
                        s_ps,
                        lhsT=xa[:, j, :],
                        rhs=diag[:, j, :],
                        start=(j == 0),
                        stop=(j == J - 1),
                    )
                s_sb = small.tile([C, P], bf16)
                nc.scalar.copy(s_sb, s_ps)

                # o[p, c'] = (sum_c s[c, p] * wvT[c, c']) / sum_e[p]
                o_ps = psum.tile([P, C], f32)
                nc.tensor.matmul(o_ps, lhsT=s_sb[:], rhs=wvt_sb[:], start=True, stop=True)
                o_sb = small.tile([P, C], f32)
                nc.scalar.mul(o_sb, o_ps, inv[:])

                nc.sync.dma_start(out=out[r0 : r0 + P, :], in_=o_sb[:])

    nc.compile()
    return nc


def _get_nc():
    if "nc" not in _cache:
        _cache["nc"] = _build()
    return _cache["nc"]


def kernel(fea_center, fea_near, wq, wk, wv):
    global last_exec_ns, last_results
    import ml_dtypes

    from concourse.bass_utils import run_bass_kernel_spmd

    bf = ml_dtypes.bfloat16
    fea_center = np.asarray(fea_center, dtype=np.float32)
    fea_near = np.asarray(fea_near, dtype=np.float32)
    wq = np.asarray(wq, dtype=np.float32)
    wk = np.asarray(wk, dtype=np.float32)
    wv = np.asarray(wv, dtype=np.float32)

    amat = np.ascontiguousarray(wq.T @ wk).astype(bf)  # [c_center, c_near]
    wvt = np.ascontiguousarray(wv.T).astype(bf)  # [c_in, c_out]

    # [bs, n, 17, c]: near neighbors then the center as the 17th entry
    xfull = np.concatenate([fea_near, fea_center], axis=2).astype(bf)
    # transposed center features [bs, c, n]
    fcT = np.ascontiguousarray(np.transpose(fea_center[:, :, 0, :], (0, 2, 1))).astype(bf)

    # local_scatter index tables: partition p scatters e[p, j] to j*128 + p
    pp = np.arange(P, dtype=np.int16)[:, None]
    jj0 = np.arange(8, dtype=np.int16)[None, :]
    sidx0 = np.ascontiguousarray(jj0 * P + pp)  # [P, 8]
    jj1 = np.arange(9, dtype=np.int16)[None, :]
    sidx1 = np.concatenate(
        [jj1 * P + pp, np.full((P, 1), -1, dtype=np.int16)], axis=1
    )  # [P, 10], last col ignored

    nc = _get_nc()
    in_maps = []
    for b in range(BS):
        in_maps.append(
            {
                "xfull": np.ascontiguousarray(xfull[b]),
                "fcT": np.ascontiguousarray(fcT[b]),
                "amat": amat,
                "wvt": wvt,
                "sidx0": sidx0,
                "sidx1": sidx1,
            }
        )

    trace = bool(int(os.environ.get("BASS_KERNEL_TRACE", "0")))
    res = run_bass_kernel_spmd(nc, in_maps, core_ids=list(range(BS)), trace=trace)
    last_exec_ns = res.exec_time_ns
    last_results = res
    out = np.stack([res.results[b]["out"] for b in range(BS)], axis=0)
    return out


# revision 25
# speedup vs baseline: 3.1724x; 1.0671x over previous
"""Trainium2 Bass kernel for nn_AttentionInPnts (sparse local attention over points).

Math (per batch b, point n):
  q = wq @ xc, k_j = wk @ x_j, v_j = wv @ x_j   (x_16 == xc, the center)
  logit_j = (q . k_j) / 8 = xc^T (wq^T wk) x_j / 8 = y . x_j / 8
  a = softmax(logit)                            (17 entries)
  out = sum_j a_j v_j = wv @ (sum_j a_j x_j)    (projection commutes with the
                                                 scalar-weighted sum)

Per point-tile of 128 points: one small PE matmul for y, one DVE mul +
reduce for the 17 logits, a softmax on [128, 17], then the TensorEngine
computes the weighted sum via the diagonal-rhs trick:
  s[c, p] = sum_j matmul(lhsT = x_j[p', c], rhs = diag(a_j)[p', p])  (PSUM accum)
and one final matmul applies wv. The diag tensors are built on the otherwise
idle GpSimd engine with affine_select.

Host-side prep (cheap, numpy): concatenate near+center, pre-transpose the
center, precompute A = wq^T wk and wv^T, and cast the streamed operands to
bf16 (halves HBM traffic; accumulations stay fp32 in PSUM / DVE internals).

Sharding: pure data-parallel, batch b -> core b (8 batches, 8 cores).
"""

import os

import numpy as np

BS = 8
NPTS = 4096
KNB = 16
C = 128
J = KNB + 1  # 16 near + 1 center
P = 128  # points per tile
NTILES = NPTS // P
SCALE = 1.0 / 8.0  # 1/sqrt(c//2)

_cache = {}

# set by kernel() when tracing is enabled (BASS_KERNEL_TRACE=1)
last_exec_ns = None
last_results = None


def _build():
    import concourse.bass as bass
    import concourse.tile as tile
    from concourse import bacc, mybir

    f32 = mybir.dt.float32
    bf16 = mybir.dt.bfloat16
    nc = bacc.Bacc()

    xfull = nc.declare_dram_parameter("xfull", [NPTS, J, C], bf16, isOutput=False)
    fcT = nc.declare_dram_parameter("fcT", [C, NPTS], bf16, isOutput=False)
    amat = nc.declare_dram_parameter("amat", [C, C], bf16, isOutput=False)
    wvt = nc.declare_dram_parameter("wvt", [C, C], bf16, isOutput=False)
    i16 = mybir.dt.int16
    sidx0 = nc.declare_dram_parameter("sidx0", [P, 8], i16, isOutput=False)
    sidx1 = nc.declare_dram_parameter("sidx1", [P, 10], i16, isOutput=False)
    out = nc.declare_dram_parameter("out", [NPTS, C], f32, isOutput=True)

    with tile.TileContext(nc) as tc:
        with (
            tc.tile_pool(name="consts", bufs=1) as consts,
            tc.tile_pool(name="big", bufs=4) as big,
            tc.tile_pool(name="small", bufs=6) as small,
            tc.tile_pool(name="psum", bufs=2, space="PSUM") as psum,
        ):
            amat_sb = consts.tile([C, C], bf16)
            nc.sync.dma_start(out=amat_sb, in_=amat[:])
            wvt_sb = consts.tile([C, C], bf16)
            nc.sync.dma_start(out=wvt_sb, in_=wvt[:])
            sidx0_sb = consts.tile([P, 8], i16)
            nc.sync.dma_start(out=sidx0_sb, in_=sidx0[:])
            sidx1_sb = consts.tile([P, 10], i16)
            nc.sync.dma_start(out=sidx1_sb, in_=sidx1[:])
            fcT_sb = consts.tile([C, NPTS], bf16)
            nc.sync.dma_start(out=fcT_sb, in_=fcT[:])

            for it in range(NTILES):
                r0 = it * P
                # xa = [p, j, c] bf16 with j=0..15 near, j=16 center
                # (two dma_starts -> two HW queues in flight; Bacc splits the
                # multi-sem waits for PE consumers)
                xa = big.tile([P, J, C], bf16)
                nc.sync.dma_start(out=xa[:], in_=xfull[r0 : r0 + P, :, :])
                # y[p, c'] = sum_c xc[p, c] * A[c, c']  (fp32 accumulate)
                y_ps = psum.tile([P, C], f32)
                nc.tensor.matmul(
                    y_ps, lhsT=fcT_sb[:, r0 : r0 + P], rhs=amat_sb[:], start=True, stop=True
                )
                y_sb = small.tile([P, C], bf16)
                nc.scalar.copy(y_sb, y_ps)

                # t[p, j, c] = xa * y (y broadcast over j), bf16 2x mode
                y_ap = y_sb[:]
                y_bc = bass.AP(
                    tensor=y_ap.tensor,
                    offset=y_ap.offset,
                    ap=[y_ap.ap[0], [0, J], y_ap.ap[1]],
                )
                t = big.tile([P, J, C], bf16)
                nc.vector.tensor_tensor(out=t[:], in0=xa[:], in1=y_bc, op=mybir.AluOpType.mult)

                # logits L[p, j] = sum_c t; DVE reduces j=0..13 (1x), the
                # Scalar engine picks up j=14..16 via activation accum_out
                JDVE = 14
                logit = small.tile([P, J], f32)
                nc.vector.tensor_reduce(
                    out=logit[:, 0:JDVE], in_=t[:, 0:JDVE, :],
                    axis=mybir.AxisListType.X, op=mybir.AluOpType.add,
                )
                act_scratch = small.tile([P, C], bf16)
                for j in range(JDVE, J):
                    nc.scalar.activation(
                        out=act_scratch[:],
                        in_=t[:, j, :],
                        func=mybir.ActivationFunctionType.Identity,
                        accum_out=logit[:, j : j + 1],
                    )

                # e = exp(L/8) in bf16, sum_e = sum_j e in fp32; the softmax
                # normalization is folded into the final output copy (x 1/sum_e)
                e_sb = small.tile([P, J + 1], bf16)
                nc.gpsimd.memset(e_sb[:, J : J + 1], 0.0)
                sum_e = small.tile([P, 1], f32)
                nc.scalar.activation(
                    out=e_sb[:, 0:J],
                    in_=logit[:],
                    func=mybir.ActivationFunctionType.Exp,
                    scale=SCALE,
                    accum_out=sum_e[:],
                )
                inv = small.tile([P, 1], f32)
                nc.vector.reciprocal(inv[:], sum_e[:])

                # diag[p', j, p] = (p' == p) * e[p', j] via GpSimd local_scatter
                # (zeros + per-partition scatter of the 17 diagonal values; the
                # int16 index tables are tile-invariant constants from the host)
                diag = big.tile([P, J, P], bf16)
                nc.gpsimd.local_scatter(
                    out_ap=diag[:, 0:8, :],
                    data_ap=e_sb[:, 0:8],
                    idxs_ap=sidx0_sb[:],
                    channels=P,
                    num_elems=8 * P,
                    num_idxs=8,
                )
                nc.gpsimd.local_scatter(
                    out_ap=diag[:, 8:J, :],
                    data_ap=e_sb[:, 8 : J + 1],
                    idxs_ap=sidx1_sb[:],
                    channels=P,
                    num_elems=9 * P,
                    num_idxs=10,
                )

                # s[c, p] = sum_j x_j[p', c]^T @ diag_j[p', p]  (PSUM accumulate)
                s_ps = psum.tile([C, P], f32)
                for j in range(J):
                    nc.tensor.matmul(
                        s_ps,
                        lhsT=xa[:, j, :],
                        rhs=diag[:, j, :],
                        start=(j == 0),
                        stop=(j == J - 1),
                    )
                s_sb = small.tile([C, P], bf16)
                nc.scalar.copy(s_sb, s_ps)

                # o[p, c'] = (sum_c s[c, p] * wvT[c, c']) / sum_e[p]
                o_ps = psum.tile([P, C], f32)
                nc.tensor.matmul(o_ps, lhsT=s_sb[:], rhs=wvt_sb[:], start=True, stop=True)
                o_sb = small.tile([P, C], f32)
                nc.scalar.mul(o_sb, o_ps, inv[:])

                nc.sync.dma_start(out=out[r0 : r0 + P, :], in_=o_sb[:])

    nc.compile()
    return nc


def _get_nc():
    if "nc" not in _cache:
        _cache["nc"] = _build()
    return _cache["nc"]


def kernel(fea_center, fea_near, wq, wk, wv):
    global last_exec_ns, last_results
    import ml_dtypes

    from concourse.bass_utils import run_bass_kernel_spmd

    bf = ml_dtypes.bfloat16
    fea_center = np.asarray(fea_center, dtype=np.float32)
    fea_near = np.asarray(fea_near, dtype=np.float32)
    wq = np.asarray(wq, dtype=np.float32)
    wk = np.asarray(wk, dtype=np.float32)
    wv = np.asarray(wv, dtype=np.float32)

    amat = np.ascontiguousarray(wq.T @ wk).astype(bf)  # [c_center, c_near]
    wvt = np.ascontiguousarray(wv.T).astype(bf)  # [c_in, c_out]

    # [bs, n, 17, c]: near neighbors then the center as the 17th entry
    xfull = np.concatenate([fea_near, fea_center], axis=2).astype(bf)
    # transposed center features [bs, c, n]
    fcT = np.ascontiguousarray(np.transpose(fea_center[:, :, 0, :], (0, 2, 1))).astype(bf)

    # local_scatter index tables: partition p scatters e[p, j] to j*128 + p
    pp = np.arange(P, dtype=np.int16)[:, None]
    jj0 = np.arange(8, dtype=np.int16)[None, :]
    sidx0 = np.ascontiguousarray(jj0 * P + pp)  # [P, 8]
    jj1 = np.arange(9, dtype=np.int16)[None, :]
    sidx1 = np.concatenate(
        [jj1 * P + pp, np.full((P, 1), -1, dtype=np.int16)], axis=1
    )  # [P, 10], last col ignored

    nc = _get_nc()
    in_maps = []
    for b in range(BS):
        in_maps.append(
            {
                "xfull": np.ascontiguousarray(xfull[b]),
                "fcT": np.ascontiguousarray(fcT[b]),
                "amat": amat,
                "wvt": wvt,
                "sidx0": sidx0,
                "sidx1": sidx1,
            }
        )

    trace = bool(int(os.environ.get("BASS_KERNEL_TRACE", "0")))
    res = run_bass_kernel_spmd(nc, in_maps, core_ids=list(range(BS)), trace=trace)
    last_exec_ns = res.exec_time_ns
    last_results = res
    out = np.stack([res.results[b]["out"] for b in range(BS)], axis=0)
    return out
